# revision 4
# baseline (speedup 1.0000x reference)
"""2-layer GraphSAGE GNN (segment_sum message passing) on 8 Trainium2 NeuronCores.

Strategy:
  - dst nodes are block-partitioned across 8 cores (core c owns original ids
    [c*N/8, (c+1)*N/8)); incident edges are partitioned by dst.
  - Within a core, dst nodes are bin-packed into tiles of <=128 nodes such
    that each (tile, src-quartile) holds at most 512 edge slots -> a fully
    static, data-independent SPMD instruction schedule across all 8 cores.
  - Edge-source rows are fetched with dma_gather (256B rows, 4 SWDGE queues);
    the segment-sum is a PE matmul against one-hot selection matrices built
    on the fly by the vector engine (is_equal vs an iota matrix).
  - Layer 1 aggregates raw x (50->64 padded); layer 2 aggregates p = h @ W2_l
    (transform-first), with p shards exchanged via an on-device AllGather.
  - log_softmax epilogue on-device; host only reassembles/permutes rows.
"""
import os
import sys
import types
import contextlib
import ctypes

import numpy as np

# ---------------------------------------------------------------- axon shim
_SO_PATH = "/opt/axon/libaxon_pjrt.so"


def _install_axon_hooks_shim():
    if "antenv.axon_hooks" in sys.modules:
        return
    try:
        lib = ctypes.CDLL(_SO_PATH)
        has = hasattr(lib, "axon_start_nrt_profile")
    except OSError:
        has = False
    if has:
        lib.axon_start_nrt_profile.argtypes = [ctypes.POINTER(ctypes.c_int64), ctypes.c_size_t]
        lib.axon_start_nrt_profile.restype = ctypes.c_int64
        lib.axon_stop_nrt_profile.argtypes = [ctypes.c_char_p]
        lib.axon_stop_nrt_profile.restype = ctypes.c_int64

        @contextlib.contextmanager
        def _hook(output_dir, device_ids):
            import jax

            jax.devices()
            if device_ids:
                ids = (ctypes.c_int64 * len(device_ids))(*device_ids)
                rc = lib.axon_start_nrt_profile(ids, len(device_ids))
            else:
                rc = lib.axon_start_nrt_profile(None, 0)
            if rc != 0:
                raise RuntimeError(f"axon_start_nrt_profile rc={rc}")
            try:
                yield
            finally:
                n = lib.axon_stop_nrt_profile(str(output_dir).encode())
                print(f"ntff profile: {n} file(s) written to {output_dir}", file=sys.stderr)

        hook = _hook
    else:
        hook = None
    mod = types.ModuleType("antenv.axon_hooks")
    mod.get_axon_ntff_profile_hook = lambda: hook
    mod.set_axon_ntff_profile_hook = lambda h: None
    sys.modules["antenv.axon_hooks"] = mod


_install_axon_hooks_shim()

import concourse.bass as bass  # noqa: E402
import concourse.mybir as mybir  # noqa: E402
import concourse.tile as tile  # noqa: E402
import concourse.bacc as bacc  # noqa: E402

NCORES = 8
CSLOT = 512          # edge slots per (tile, quartile)
CPT = CSLOT // 128   # chunks (of 128 edges) per (tile, quartile)
GROUP = int(os.environ.get("KERNEL_GROUP", "16"))  # tiles per group
NQUEUES = int(os.environ.get("KERNEL_NQ", "4"))
F32 = mybir.dt.float32
BF16 = mybir.dt.bfloat16
I16 = mybir.dt.int16
HP = 64              # padded feature width (gather row = 256B)


# ------------------------------------------------------------ host: packing
def _pack_core(deg_q, nt):
    """First-fit-decreasing pack of dsts into nt tiles with per-quartile
    capacity CSLOT and <=128 dsts per tile. deg_q: [B, 4] int64.
    Returns (tile_of[B], pos_of[B]) or None."""
    B = deg_q.shape[0]
    order = np.argsort(-deg_q.sum(1), kind="stable")
    cap = np.full((nt, 4), CSLOT, np.int64)
    cnt = np.zeros(nt, np.int64)
    tile_of = np.full(B, -1, np.int64)
    pos_of = np.full(B, -1, np.int64)
    for d in order:
        v = deg_q[d]
        ok = (cap >= v).all(axis=1) & (cnt < 128)
        t = int(np.argmax(ok))
        if not ok[t]:
            return None
        cap[t] -= v
        tile_of[d] = t
        pos_of[d] = cnt[t]
        cnt[t] += 1
    return tile_of, pos_of


def _prepare(x, src, dst):
    """All index prep. Returns per-core arrays + global metadata."""
    N = x.shape[0]
    E = src.shape[0]
    assert N % NCORES == 0
    BLOCK = N // NCORES
    QS = N // 4                  # L1 quartile size (2 cores per quartile)
    assert QS % BLOCK == 0 and QS // BLOCK == 2
    src = np.asarray(src, np.int64)
    dst = np.asarray(dst, np.int64)

    core_of_dst = dst // BLOCK
    q_of_src = src // QS

    key = dst * 4 + q_of_src
    degq_flat = np.bincount(key, minlength=N * 4).reshape(N, 4)

    per_core_q = np.zeros((NCORES, 4), np.int64)
    for c in range(NCORES):
        per_core_q[c] = degq_flat[c * BLOCK:(c + 1) * BLOCK].sum(0)
    nt_min = int(np.ceil(per_core_q.max() / CSLOT))
    nt = max(int(np.ceil(nt_min * 1.05)), nt_min + 1, 2)
    nt = min(nt, 128)

    while True:
        packs = []
        ok = True
        for c in range(NCORES):
            r = _pack_core(degq_flat[c * BLOCK:(c + 1) * BLOCK], nt)
            if r is None:
                ok = False
                break
            packs.append(r)
        if ok:
            break
        nt += 2
        assert nt <= 128, "packing failed up to NT=128"

    NTC = nt * 128               # padded rows per core
    QS2 = 2 * NTC                # L2 quartile size in p-table rows
    assert QS2 <= 32768

    tile_all = np.empty(N, np.int64)
    pos_all = np.empty(N, np.int64)
    for c in range(NCORES):
        t_of, p_of = packs[c]
        tile_all[c * BLOCK:(c + 1) * BLOCK] = t_of
        pos_all[c * BLOCK:(c + 1) * BLOCK] = p_of
    row_all = (np.arange(N) // BLOCK) * NTC + tile_all * 128 + pos_all
    assert (row_all // QS2 == np.arange(N) // QS).all()

    ngroups = (nt + GROUP - 1) // GROUP
    gsz = [min(GROUP, nt - g * GROUP) for g in range(ngroups)]
    gsz_arr = np.asarray(gsz, np.int64)
    S = nt * 4 * CSLOT           # total slots per core
    gbase = np.cumsum([0] + [gs * 4 * CSLOT for gs in gsz])

    def stream_pos(t, q, k):
        g = t // GROUP
        ti = t % GROUP
        return gbase[g] + q * (gsz_arr[g] * CSLOT) + ti * CSLOT + k

    cores = []
    for c in range(NCORES):
        sel = core_of_dst == c
        s_c = src[sel]
        d_c = dst[sel] - c * BLOCK
        q_c = q_of_src[sel]
        t_c = tile_all[c * BLOCK + d_c]
        order = np.lexsort((q_c, t_c))
        s_s, q_s, t_s = s_c[order], q_c[order], t_c[order]
        d_s = d_c[order]
        tq = t_s * 4 + q_s
        change = np.empty(len(tq), bool)
        change[0] = True
        change[1:] = tq[1:] != tq[:-1]
        run_id = np.cumsum(change) - 1
        starts = np.flatnonzero(change)
        k = np.arange(len(tq)) - starts[run_id]
        assert k.max(initial=0) < CSLOT
        pos = stream_pos(t_s, q_s, k)

        idx1 = np.zeros(S, np.int16)
        idx1[pos] = (s_s - q_s * QS).astype(np.int16)
        dl = np.full(S, -1.0, np.float32)
        dl[pos] = pos_all[c * BLOCK + d_s].astype(np.float32)
        r2 = row_all[s_s]
        q2 = r2 // QS2
        assert (q2 == q_s).all()
        idx2 = np.zeros(S, np.int16)
        idx2[pos] = (r2 - q2 * QS2).astype(np.int16)

        def wrap16(a):
            w = a.reshape(-1, 16).T
            return np.tile(w, (8, 1)).copy()

        dlw = dl.reshape(-1, 128).T.copy()

        xT = np.zeros((x.shape[1], NTC), np.float32)
        nodes = np.arange(c * BLOCK, (c + 1) * BLOCK)
        cols = tile_all[nodes] * 128 + pos_all[nodes]
        xT[:, cols] = x[nodes].T

        cores.append(dict(idx1=wrap16(idx1), idx2=wrap16(idx2), dl=dlw, xT=xT))

    meta = dict(N=N, E=E, BLOCK=BLOCK, QS=QS, QS2=QS2, NT=nt, NTC=NTC,
                ngroups=ngroups, gsz=gsz, gbase=gbase, S=S,
                tile_all=tile_all, pos_all=pos_all)
    return cores, meta


# ------------------------------------------------------------ device program
def build_program(meta, IN_FEAT, HIDDEN, NCLS):
    NT, NTC, QS, QS2 = meta["NT"], meta["NTC"], meta["QS"], meta["QS2"]
    ngroups, gsz, gbase, S = meta["ngroups"], meta["gsz"], meta["gbase"], meta["S"]
    N = meta["N"]
    assert IN_FEAT <= HP and HIDDEN <= HP and NCLS <= HP

    nc = bacc.Bacc("TRN2", target_bir_lowering=False, num_devices=NCORES,
                   num_swdge_queues=NQUEUES,
                   dynamic_dma_scratch_size=int(os.environ.get("KERNEL_SCRATCH", "32768")))

    t_x1 = nc.dram_tensor("x1pad", [N, HP], F32, kind="ExternalInput")
    t_xT = nc.dram_tensor("xT", [IN_FEAT, NTC], F32, kind="ExternalInput")
    t_idx1 = nc.dram_tensor("idx1", [128, S // 16], I16, kind="ExternalInput")
    t_idx2 = nc.dram_tensor("idx2", [128, S // 16], I16, kind="ExternalInput")
    t_dl = nc.dram_tensor("dl", [128, S // 128], F32, kind="ExternalInput")
    t_w1l = nc.dram_tensor("w1l", [HP, HIDDEN], F32, kind="ExternalInput")
    t_w1r = nc.dram_tensor("w1r", [IN_FEAT, HIDDEN], F32, kind="ExternalInput")
    t_w2l = nc.dram_tensor("w2l", [HIDDEN, HP], BF16, kind="ExternalInput")
    t_w2r = nc.dram_tensor("w2r", [HIDDEN, NCLS], BF16, kind="ExternalInput")
    t_b1 = nc.dram_tensor("b1c", [HIDDEN, 1], F32, kind="ExternalInput")
    t_b2 = nc.dram_tensor("b2r", [128, NCLS], F32, kind="ExternalInput")
    t_iota = nc.dram_tensor("iota", [128, 128], F32, kind="ExternalInput")
    t_out = nc.dram_tensor("out", [NTC, NCLS], F32, kind="ExternalOutput")

    AluOp = mybir.AluOpType
    Act = mybir.ActivationFunctionType

    with tile.TileContext(nc) as tc:
        with (
            tc.tile_pool(name="const", bufs=1) as constp,
            tc.tile_pool(name="ht", bufs=1) as htp,
            tc.tile_pool(name="gbuf", bufs=int(os.environ.get("KERNEL_GBUF", "4"))) as gp,
            tc.tile_pool(name="idxp", bufs=int(os.environ.get("KERNEL_IDXB", "4"))) as idxp,
            tc.tile_pool(name="ohp", bufs=int(os.environ.get("KERNEL_OHB", "4"))) as ohp,
            tc.tile_pool(name="sb1", bufs=1) as sb1p,
            tc.tile_pool(name="sb2", bufs=2) as sb2p,
            tc.tile_pool(name="dram", bufs=1, space="DRAM") as dramp,
        ):
            iota_sb = constp.tile([128, 128], F32)
            nc.sync.dma_start(iota_sb[:], t_iota.ap())
            w1l_sb = constp.tile([HP, HIDDEN], F32)
            nc.sync.dma_start(w1l_sb[:], t_w1l.ap())
            w1r_sb = constp.tile([IN_FEAT, HIDDEN], F32)
            nc.sync.dma_start(w1r_sb[:], t_w1r.ap())
            w2l_sb = constp.tile([HIDDEN, HP], BF16)
            nc.sync.dma_start(w2l_sb[:], t_w2l.ap())
            w2r_sb = constp.tile([HIDDEN, NCLS], BF16)
            nc.sync.dma_start(w2r_sb[:], t_w2r.ap())
            b1_sb = constp.tile([HIDDEN, 1], F32)
            nc.sync.dma_start(b1_sb[:], t_b1.ap())
            b2_sb = constp.tile([128, NCLS], F32)
            nc.sync.dma_start(b2_sb[:], t_b2.ap())

            hT = htp.tile([HIDDEN, NTC], BF16)
            p_shard = dramp.tile([NTC, HP], F32)
            p_full = dramp.tile([NCORES * NTC, HP], F32)

            def load_group_meta(g, t_idx):
                Gs = gsz[g]
                base = int(gbase[g])
                callsz = Gs * CSLOT
                idx_g = idxp.tile([128, GROUP * 4 * CSLOT // 16], I16,
                                  tag="idxg", name="idxg")
                nc.sync.dma_start(idx_g[:, :4 * callsz // 16],
                                  t_idx.ap()[:, base // 16:(base + 4 * callsz) // 16])
                dl_g = idxp.tile([128, GROUP * 4 * CPT], F32, tag="dlg", name="dlg")
                nc.sync.dma_start(dl_g[:, :4 * callsz // 128],
                                  t_dl.ap()[:, base // 128:(base + 4 * callsz) // 128])
                return idx_g, dl_g

            def gather_group(g, idx_g, table_ap, qsize):
                Gs = gsz[g]
                callsz = Gs * CSLOT
                gts = []
                for q in range(4):
                    gt = gp.tile([128, GROUP * CPT, HP], F32, tag="gt", name="gt")
                    nc.gpsimd.dma_gather(
                        gt[:, :Gs * CPT, :],
                        table_ap[q * qsize:(q + 1) * qsize, :],
                        idx_g[:, q * callsz // 16:(q + 1) * callsz // 16],
                        callsz, callsz, HP, elem_step=HP,
                        single_packet=os.environ.get("KERNEL_SP", "0") == "1",
                        queue_num=q % NQUEUES)
                    gts.append(gt)
                return gts

            def build_oh(dl_g, nchunk, q, j0, nj):
                """One-hot for chunks [j0, j0+nj) of call q."""
                oh = ohp.tile([128, 2 * CPT, 128], F32, tag=f"oh{q}", name=f"oh{q}")
                in0 = dl_g[:, q * nchunk + j0: q * nchunk + j0 + nj]
                in0 = in0.unsqueeze(2).to_broadcast([128, nj, 128])
                in1 = iota_sb[:].unsqueeze(1).to_broadcast([128, nj, 128])
                nc.vector.tensor_tensor(oh[:, :nj, :], in0, in1, op=AluOp.is_equal)
                return oh

            # ---------------- Layer 1 ----------------
            with (
                tc.tile_pool(name="psA", bufs=4, space="PSUM") as psAp,
                tc.tile_pool(name="psz", bufs=2, space="PSUM") as pszp,
                tc.tile_pool(name="psp", bufs=2, space="PSUM") as pspp,
            ):
                for g in range(ngroups):
                    Gs = gsz[g]
                    nchunk = Gs * CPT
                    idx_g, dl_g = load_group_meta(g, t_idx1)
                    gts = gather_group(g, idx_g, t_x1.ap(), QS)
                    a1sb = sb1p.tile([HP, GROUP * 128], F32, tag="a1sb", name="a1sb")
                    ohs = [None] * 4
                    for ti in range(Gs):
                        if ti % 2 == 0:
                            nj = min(2 * CPT, nchunk - ti * CPT)
                            ohs = [build_oh(dl_g, nchunk, q, ti * CPT, nj)
                                   for q in range(4)]
                        psa = psAp.tile([HP, 128], F32, tag="psa", name="psa")
                        for q in range(4):
                            for cc in range(CPT):
                                j = ti * CPT + cc
                                nc.tensor.matmul(
                                    psa[:],
                                    lhsT=gts[q][:, j, :],
                                    rhs=ohs[q][:, (ti % 2) * CPT + cc, :],
                                    start=(q == 0 and cc == 0),
                                    stop=(q == 3 and cc == CPT - 1))
                        nc.scalar.copy(a1sb[:, ti * 128:(ti + 1) * 128], psa[:])
                    xT_g = sb1p.tile([IN_FEAT, GROUP * 128], F32, tag="xtg", name="xtg")
                    nc.sync.dma_start(
                        xT_g[:, :Gs * 128],
                        t_xT.ap()[:, g * GROUP * 128: g * GROUP * 128 + Gs * 128])
                    for sub in range((Gs + 3) // 4):
                        w = min(4, Gs - sub * 4) * 128
                        c0 = sub * 512
                        z1 = pszp.tile([HIDDEN, 512], F32, tag="z1", name="z1")
                        nc.tensor.matmul(z1[:, :w], lhsT=w1l_sb[:],
                                         rhs=a1sb[:, c0:c0 + w], start=True, stop=False)
                        nc.tensor.matmul(z1[:, :w], lhsT=w1r_sb[:],
                                         rhs=xT_g[:, c0:c0 + w], start=False, stop=True)
                        h0 = g * GROUP * 128 + c0
                        nc.scalar.activation(hT[:, h0:h0 + w], z1[:, :w],
                                             Act.Relu, bias=b1_sb[:, :1])
                        pp = pspp.tile([128, 4 * HP], F32, tag="pp", name="pp")
                        for k in range(w // 128):
                            tg = g * GROUP + sub * 4 + k
                            nc.tensor.matmul(pp[:, k * HP:(k + 1) * HP],
                                             lhsT=hT[:, tg * 128:(tg + 1) * 128],
                                             rhs=w2l_sb[:], start=True, stop=True)
                        psb = sb2p.tile([128, 4 * HP], F32, tag="psb", name="psb")
                        nc.scalar.copy(psb[:, :w // 2], pp[:, :w // 2])
                        rows0 = (g * GROUP + sub * 4) * 128
                        out_ap = p_shard[rows0:rows0 + w, :].rearrange(
                            "(t p) c -> p t c", p=128)
                        nc.sync.dma_start(out_ap, psb[:, :w // 2].rearrange(
                            "p (t c) -> p t c", c=HP))

            nc.gpsimd.collective_compute(
                "AllGather", AluOp.bypass,
                replica_groups=[list(range(NCORES))],
                ins=[p_shard.opt()],
                outs=[p_full.opt()],
            )

            # ---------------- Layer 2 ----------------
            with (
                tc.tile_pool(name="pso", bufs=5, space="PSUM") as psop,
                tc.tile_pool(name="smp", bufs=2) as smp,
            ):
                for g in range(ngroups):
                    Gs = gsz[g]
                    nchunk = Gs * CPT
                    idx_g, dl_g = load_group_meta(g, t_idx2)
                    gts = gather_group(g, idx_g, p_full[:, :], QS2)
                    o_sb = sb2p.tile([128, GROUP * HP], F32, tag="osb", name="osb")
                    ohs = [None] * 4
                    for ti in range(Gs):
                        if ti % 2 == 0:
                            nj = min(2 * CPT, nchunk - ti * CPT)
                            ohs = [build_oh(dl_g, nchunk, q, ti * CPT, nj)
                                   for q in range(4)]
                        po = psop.tile([128, HP], F32, tag="po", name="po")
                        for q in range(4):
                            for cc in range(CPT):
                                j = ti * CPT + cc
                                nc.tensor.matmul(
                                    po[:],
                                    lhsT=ohs[q][:, (ti % 2) * CPT + cc, :],
                                    rhs=gts[q][:, j, :],
                                    start=(q == 0 and cc == 0), stop=False)
                        tg = g * GROUP + ti
                        nc.tensor.matmul(po[:, :NCLS],
                                         lhsT=hT[:, tg * 128:(tg + 1) * 128],
                                         rhs=w2r_sb[:], start=False, stop=True)
                        nc.scalar.copy(o_sb[:, ti * HP:(ti + 1) * HP], po[:])
                    # log_softmax over classes for the whole group
                    o3 = o_sb[:, :Gs * HP].rearrange("p (t c) -> p t c", c=HP)[:, :, :NCLS]
                    ob = smp.tile([128, GROUP, NCLS], F32, tag="ob", name="ob")
                    nc.vector.tensor_tensor(
                        ob[:, :Gs, :], o3,
                        b2_sb[:].unsqueeze(1).to_broadcast([128, Gs, NCLS]),
                        op=AluOp.add)
                    mx = smp.tile([128, GROUP], F32, tag="mx", name="mx")
                    nc.vector.tensor_reduce(mx[:, :Gs], ob[:, :Gs, :],
                                            axis=mybir.AxisListType.X, op=AluOp.max)
                    tmp = smp.tile([128, GROUP, NCLS], F32, tag="tmp", name="tmp")
                    nc.vector.tensor_tensor(
                        tmp[:, :Gs, :], ob[:, :Gs, :],
                        mx[:, :Gs].unsqueeze(2).to_broadcast([128, Gs, NCLS]),
                        op=AluOp.subtract)
                    ex = smp.tile([128, GROUP, NCLS], F32, tag="ex", name="ex")
                    nc.scalar.activation(ex[:, :Gs, :], tmp[:, :Gs, :], Act.Exp)
                    sm = smp.tile([128, GROUP], F32, tag="sm", name="sm")
                    nc.vector.tensor_reduce(sm[:, :Gs], ex[:, :Gs, :],
                                            axis=mybir.AxisListType.X, op=AluOp.add)
                    ls = smp.tile([128, GROUP], F32, tag="ls", name="ls")
                    nc.scalar.activation(ls[:, :Gs], sm[:, :Gs], Act.Ln)
                    ov = smp.tile([128, GROUP, NCLS], F32, tag="ov", name="ov")
                    nc.vector.tensor_tensor(
                        ov[:, :Gs, :], tmp[:, :Gs, :],
                        ls[:, :Gs].unsqueeze(2).to_broadcast([128, Gs, NCLS]),
                        op=AluOp.subtract)
                    rows0 = g * GROUP * 128
                    out_ap = t_out.ap()[rows0:rows0 + Gs * 128, :].rearrange(
                        "(t p) c -> p t c", p=128)
                    nc.sync.dma_start(out_ap, ov[:, :Gs, :])

    nc.compile()
    return nc


# ------------------------------------------------------------ runner (PJRT)
class _Runner:
    def __init__(self, nc, n_cores):
        import jax
        from jax.sharding import Mesh, PartitionSpec
        from jax.experimental.shard_map import shard_map
        from concourse.bass2jax import (_bass_exec_p, install_neuronx_cc_hook,
                                        partition_id_tensor)

        install_neuronx_cc_hook()
        self.n_cores = n_cores
        in_names, out_names, out_avals, zero_outs = [], [], [], []
        partition_name = nc.partition_id_tensor.name if nc.partition_id_tensor else None
        for alloc in nc.m.functions[0].allocations:
            if not isinstance(alloc, mybir.MemoryLocationSet):
                continue
            name = alloc.memorylocations[0].name
            if alloc.kind == "ExternalInput":
                if name != partition_name:
                    in_names.append(name)
            elif alloc.kind == "ExternalOutput":
                shape = tuple(alloc.tensor_shape)
                dtype = mybir.dt.np(alloc.dtype)
                out_names.append(name)
                out_avals.append(jax.core.ShapedArray(shape, dtype))
                zero_outs.append(np.zeros(shape, dtype))
        n_params = len(in_names)
        n_outs = len(out_avals)
        all_in = list(in_names) + list(out_names)
        if partition_name is not None:
            all_in.append(partition_name)
        self.in_names, self.out_names, self.zero_outs = in_names, out_names, zero_outs
        donate = tuple(range(n_params, n_params + n_outs))

        def _body(*args):
            operands = list(args)
            if partition_name is not None:
                operands.append(partition_id_tensor())
            outs = _bass_exec_p.bind(
                *operands,
                out_avals=tuple(out_avals),
                in_names=tuple(all_in),
                out_names=tuple(out_names),
                lowering_input_output_aliases=(),
                sim_require_finite=True,
                sim_require_nnan=True,
                nc=nc,
            )
            return tuple(outs)

        devices = jax.devices()[:n_cores]
        self.mesh = Mesh(np.asarray(devices), ("core",))
        in_specs = (PartitionSpec("core"),) * (n_params + n_outs)
        out_specs = (PartitionSpec("core"),) * n_outs
        self.fn = jax.jit(
            shard_map(_body, mesh=self.mesh, in_specs=in_specs,
                      out_specs=out_specs, check_rep=False),
            donate_argnums=donate, keep_unused=True)

    def run(self, in_maps):
        n = self.n_cores
        concat_in = [
            np.concatenate([np.asarray(in_maps[c][nm]) for c in range(n)], axis=0)
            for nm in self.in_names
        ] + [np.concatenate([z] * n, axis=0) for z in self.zero_outs]
        outs = self.fn(*concat_in)
        results = []
        for c in range(n):
            d = {}
            for i, nm in enumerate(self.out_names):
                full = np.asarray(outs[i])
                per = full.shape[0] // n
                d[nm] = full[c * per:(c + 1) * per]
            results.append(d)
        return results


_CACHE = {}


def _get_program(meta, IN_FEAT, HIDDEN, NCLS, use_sim):
    key = (meta["NT"], meta["N"], IN_FEAT, HIDDEN, NCLS, use_sim)
    if key not in _CACHE:
        nc = build_program(meta, IN_FEAT, HIDDEN, NCLS)
        runner = None if use_sim else _Runner(nc, NCORES)
        _CACHE[key] = (nc, runner)
    return _CACHE[key]


def _make_in_maps(x, W1_l, W1_r, b1, W2_l, W2_r, b2, cores):
    N, IN_FEAT = x.shape
    HIDDEN = W1_l.shape[1]
    NCLS = W2_l.shape[1]
    x1pad = np.zeros((N, HP), np.float32)
    x1pad[:, :IN_FEAT] = x
    w1l_pad = np.zeros((HP, HIDDEN), np.float32)
    w1l_pad[:IN_FEAT] = W1_l
    w2l_pad = np.zeros((HIDDEN, HP), np.float32)
    w2l_pad[:, :NCLS] = W2_l
    iota = np.broadcast_to(np.arange(128, dtype=np.float32), (128, 128)).copy()
    b2r = np.broadcast_to(b2, (128, NCLS)).copy()
    import ml_dtypes
    w2l_bf = w2l_pad.astype(ml_dtypes.bfloat16)
    w2r_bf = W2_r.astype(ml_dtypes.bfloat16)

    in_maps = []
    for c in range(NCORES):
        in_maps.append({
            "x1pad": x1pad,
            "xT": cores[c]["xT"],
            "idx1": cores[c]["idx1"],
            "idx2": cores[c]["idx2"],
            "dl": cores[c]["dl"],
            "w1l": w1l_pad,
            "w1r": W1_r,
            "w2l": w2l_bf,
            "w2r": w2r_bf,
            "b1c": b1.reshape(HIDDEN, 1),
            "b2r": b2r,
            "iota": iota,
        })
    return in_maps


def kernel(x, W1_l, W1_r, b1, W2_l, W2_r, b2, src, dst):
    x = np.asarray(x, np.float32)
    W1_l = np.asarray(W1_l, np.float32)
    W1_r = np.asarray(W1_r, np.float32)
    b1 = np.asarray(b1, np.float32)
    W2_l = np.asarray(W2_l, np.float32)
    W2_r = np.asarray(W2_r, np.float32)
    b2 = np.asarray(b2, np.float32)

    N, IN_FEAT = x.shape
    HIDDEN = W1_l.shape[1]
    NCLS = W2_l.shape[1]

    cores, meta = _prepare(x, src, dst)

    use_sim = os.environ.get("KERNEL_SIM", "0") == "1"
    nc, runner = _get_program(meta, IN_FEAT, HIDDEN, NCLS, use_sim)

    in_maps = _make_in_maps(x, W1_l, W1_r, b1, W2_l, W2_r, b2, cores)

    if use_sim:
        from concourse.bass_interp import MultiCoreSim
        sim = MultiCoreSim(nc, num_cores=NCORES, trace=False)
        for c, core in sim.cores.items():
            core.assign_tensors(in_maps[c])
        sim.simulate(check_with_hw=False)
        results = [{"out": sim.cores[c].tensor("out").copy()} for c in range(NCORES)]
    else:
        results = runner.run(in_maps)

    out = np.empty((N, NCLS), np.float32)
    BLOCK = meta["BLOCK"]
    rows_local = meta["tile_all"] * 128 + meta["pos_all"]
    for c in range(NCORES):
        nodes = np.arange(c * BLOCK, (c + 1) * BLOCK)
        out[nodes] = results[c]["out"][rows_local[nodes]]
    return out

